# revision 5
# baseline (speedup 1.0000x reference)
"""AttentionLayer Trainium2 kernel: 8-way SPMD (batch x query-half data parallel).

Per core (b = core//2, h = core%2), with x rotated so the core's query half
occupies columns 0..2047:
  k  = wk @ x + bk            [32, 4096]
  q  = wq @ x[:, :2048] + bq  [32, 2048]
  vT = x^T @ wv^T             [4096, 256]   (v transposed, born in [j, c] layout)
  S^T[j, i] = k[:, j]^T q[:, i]   -> P = exp(S^T - 8)  (fixed shift keeps P in
                                     fp8e5 range; shift cancels in the ratio)
  out[c, i] = (sum_j vT[j, c] P[j, i]) / (sum_j P[j, i]) + x[c, i]

Matmul chains run in bf16; the PV contraction (the dominant matmul, K=4096)
runs in fp8 DoubleRow mode: P is written by the ACT exp directly as fp8e5,
vT is evacuated from PSUM as fp8e4, and each DoubleRow matmul contracts two
128-j blocks at once (3D APs [128, 2, n]) for ~2x PE throughput.

Scores PSUM is split into two [128, 1024] ping-pong halves (scA/scB) so the
next block's score matmuls overlap the current block's exp (the WAR stall on
a single 4-bank score tile was the baseline's critical path).  The softmax
denominator runs as DVE pairwise adds (bf16) + GpSimd f32 accumulation so
the vector engine stays off the critical path.
"""
import numpy as np
import ml_dtypes

import concourse.bacc as bacc
import concourse.tile as tile
from concourse import mybir
from concourse.bass_utils import run_bass_kernel_spmd

F32 = mybir.dt.float32
F32R = mybir.dt.float32r
BF16 = mybir.dt.bfloat16
F8P = mybir.dt.float8e5    # P = exp(scores - OFF): wide range, 2-bit mantissa
F8V = mybir.dt.float8e4    # vT: |v| <~ 4, fine mantissa
AF = mybir.ActivationFunctionType
ALU = mybir.AluOpType
DR = mybir.MatmulPerfMode.DoubleRow

C = 256          # channels
D = 32           # q/k dim (C // 8)
N = 4096         # h*w
NQ = 2048        # queries per core
NCORE = 8
NG = 8           # score groups per slice (4 j-blocks each)
OFF = 8.0        # exp shift: max score ~13.1 -> max P ~ e^5.1 ~ 158 (fp8e5 ok)

_cache = {}


def _build():
    nc = bacc.Bacc(None, target_bir_lowering=False)
    xb_ext = nc.declare_dram_parameter("xb", [C, N], BF16, isOutput=False)
    xres_ext = nc.declare_dram_parameter("xres", [C, NQ], F32, isOutput=False)
    wqt_ext = nc.declare_dram_parameter("wqt", [C, D], BF16, isOutput=False)
    wkt_ext = nc.declare_dram_parameter("wkt", [C, D], BF16, isOutput=False)
    wvt_ext = nc.declare_dram_parameter("wvt", [C, C], BF16, isOutput=False)
    bq4_ext = nc.declare_dram_parameter("bq4", [128, 1], F32, isOutput=False)
    bk4_ext = nc.declare_dram_parameter("bk4", [128, 1], F32, isOutput=False)
    out_ext = nc.declare_dram_parameter("out", [C, NQ], F32, isOutput=True)

    with tile.TileContext(nc) as tc:
        with (
            tc.tile_pool(name="const", bufs=1) as const,
            tc.tile_pool(name="big", bufs=1) as big,
            tc.tile_pool(name="pbuf", bufs=3) as pbuf,
            tc.tile_pool(name="work", bufs=3) as work,
            tc.tile_pool(name="accp", bufs=2) as accp,
            tc.tile_pool(name="ps_scA", bufs=1, space="PSUM") as ps_scA,
            tc.tile_pool(name="ps_scB", bufs=1, space="PSUM") as ps_scB,
            tc.tile_pool(name="ps_pv", bufs=1, space="PSUM") as ps_pv,
            tc.tile_pool(name="ps_small", bufs=1, space="PSUM") as ps_small,
            tc.tile_pool(name="ps_vt", bufs=1, space="PSUM") as ps_vt,
        ):
            wqt_sb = const.tile([128, 2 * D], BF16)
            wkt_sb = const.tile([128, 2 * D], BF16)
            wvt_sb = const.tile([128, 2 * C], BF16)
            bq4_sb = const.tile([128, 1], F32)
            bk4_sb = const.tile([128, 1], F32)
            ones_f = const.tile([128, 1], F32)
            ones_r = const.tile([128, 1], F32R)
            onesrow_f = const.tile([1, 128], F32)
            onesrow_r = const.tile([1, 128], F32R)
            negoff = const.tile([128, 1], F32)

            x_sb = big.tile([128, 2 * N], BF16)       # ci blocks side by side
            xres_sb = big.tile([128, 2 * NQ], F32)
            # k4: strip r (partitions 32r..32r+31) holds j-blocks 4g+r at
            # free g*128..(g+1)*128
            k4_sb = big.tile([128, 1024], BF16)
            # q4: strip r holds a full copy of q (slices side by side)
            q4_sb = big.tile([128, NQ], BF16)
            vt_sb = big.tile([128, 32 * C], F8V)      # [j%128, jb*256 + c]

            # critical-path DMAs first: q/k weights + the x chunks the first
            # q/k projections need; wvt + biases + rest of x on sync queue
            for ci in range(2):
                nc.scalar.dma_start(wqt_sb[:, ci * D:(ci + 1) * D],
                                    wqt_ext[ci * 128:(ci + 1) * 128, :])
                nc.scalar.dma_start(wkt_sb[:, ci * D:(ci + 1) * D],
                                    wkt_ext[ci * 128:(ci + 1) * 128, :])
            for s in range(4):
                for ci in range(2):
                    nc.scalar.dma_start(
                        x_sb[:, ci * N + s * 512: ci * N + (s + 1) * 512],
                        xb_ext[ci * 128:(ci + 1) * 128, s * 512:(s + 1) * 512])
            nc.sync.dma_start(bq4_sb[:], bq4_ext[:])
            nc.sync.dma_start(bk4_sb[:], bk4_ext[:])
            for ci in range(2):
                nc.sync.dma_start(wvt_sb[:, ci * C:(ci + 1) * C],
                                  wvt_ext[ci * 128:(ci + 1) * 128, :])
            nc.sync.dma_start(x_sb[:, 2048:4096], xb_ext[0:128, 2048:4096])
            nc.sync.dma_start(
                x_sb[:, N + 2048:2 * N], xb_ext[128:256, 2048:4096])
            nc.vector.memset(ones_f[:], 1.0)
            nc.vector.tensor_copy(ones_r[:], ones_f[:])
            nc.vector.memset(onesrow_f[:], 1.0)
            nc.vector.tensor_copy(onesrow_r[:], onesrow_f[:])
            nc.vector.memset(negoff[:], -OFF)

            def k_proj(gh):
                """Fill k4_sb[:, gh*512:(gh+1)*512] (j-blocks 16gh..16gh+15).

                Column-tiled: strip r gets blocks 4g+r, g in 4gh..4gh+3."""
                ps = ps_vt.tile([128, 512], F32, tag="vt", name="k_ps")
                for r in range(4):
                    for ci in range(2):
                        # rhs: x columns of blocks {4g+r : g in 4gh..4gh+3}
                        # block b at free offset b*128 = (4g+r)*128
                        base = ci * N + (16 * gh + r) * 128
                        rhs = x_sb[:, base: base + 13 * 128]
                        rhs = rhs.rearrange("p (g f) -> p g f", f=128)[:, 0:13:4, :]
                        nc.tensor.matmul(
                            ps[32 * r:32 * (r + 1), :],
                            wkt_sb[:, ci * D:(ci + 1) * D],
                            rhs,
                            start=(ci == 0), stop=(ci == 1),
                            tile_position=(0, 32 * r))
                nc.vector.tensor_scalar_add(
                    k4_sb[:, gh * 512:(gh + 1) * 512], ps[:], bk4_sb[:])

            def q_proj(t):
                """Fill q4_sb[:, t*512:(t+1)*512]: q slice replicated in 4 strips."""
                ps = ps_small.tile([128, 512], F32, tag="small", name="q_ps")
                for r in range(4):
                    for ci in range(2):
                        nc.tensor.matmul(
                            ps[32 * r:32 * (r + 1), :],
                            wqt_sb[:, ci * D:(ci + 1) * D],
                            x_sb[:, ci * N + t * 512: ci * N + (t + 1) * 512],
                            start=(ci == 0), stop=(ci == 1),
                            tile_position=(0, 32 * r))
                nc.vector.tensor_scalar_add(
                    q4_sb[:, t * 512:(t + 1) * 512], ps[:], bq4_sb[:])

            def vt_proj_pair(jb, pool=None, tag="vt"):
                """vT for j-blocks jb and jb+1 in one PSUM bank / one
                accumulation group (disjoint halves), one evacuation."""
                vps = (pool or ps_vt).tile([128, 2 * C], F32, tag=tag,
                                           name="vt_ps")
                for u in range(2):
                    for ci in range(2):
                        nc.tensor.matmul(
                            vps[:, u * C:(u + 1) * C],
                            x_sb[:, ci * N + (jb + u) * 128:
                                 ci * N + (jb + u + 1) * 128],
                            wvt_sb[:, ci * C:(ci + 1) * C],
                            start=(u == 0 and ci == 0),
                            stop=(u == 1 and ci == 1))
                nc.vector.tensor_copy(vt_sb[:, jb * C:(jb + 2) * C], vps[:])

            q_proj(0)
            k_proj(0)
            # vT for j-blocks 0..7: run while the rest of x lands
            vt_proj_pair(0)
            vt_proj_pair(2, pool=ps_small, tag="small")
            vt_proj_pair(4)
            vt_proj_pair(6, pool=ps_small, tag="small")
            # xres is only needed at slice epilogues: emit late so these
            # transfers don't delay the critical-path x chunks
            for t in range(4):
                for ci in range(2):
                    nc.sync.dma_start(
                        xres_sb[:, ci * NQ + t * 512: ci * NQ + (t + 1) * 512],
                        xres_ext[ci * 128:(ci + 1) * 128, t * 512:(t + 1) * 512])

            pairs = [(t, g) for t in range(4) for g in range(NG)]
            acc01 = {}
            acc23 = {}
            pvls = {}
            p_tiles = {}
            epi = {}

            def scores_half(t, g, half):
                """Score matmuls for strips (0,1) [half=0] or (2,3) [half=1]
                into a 2-bank PSUM tile, then exp into the fp8 P tile."""
                pool = ps_scA if half == 0 else ps_scB
                sc = pool.tile([128, 1024], F32, tag=f"sc{half}",
                               name=f"sc{half}")
                for rr in range(2):
                    r = 2 * half + rr
                    nc.tensor.matmul(
                        sc[:, rr * 512:(rr + 1) * 512],
                        k4_sb[32 * r:32 * (r + 1), g * 128:(g + 1) * 128],
                        q4_sb[32 * r:32 * (r + 1), t * 512:(t + 1) * 512],
                        start=True, stop=True,
                        tile_position=(32 * r, 0))
                p_sb = p_tiles[(t, g)]
                nc.scalar.activation(
                    p_sb[:, half * 1024:(half + 1) * 1024], sc[:],
                    AF.Exp, bias=negoff[:])

            def denom_half(t, g, half):
                """DVE pairwise add of the two strips of this half (bf16),
                then GpSimd accumulation into the per-t f32 accumulator."""
                p_sb = p_tiles[(t, g)]
                tmp = work.tile([128, 512], BF16, tag=f"tmp{half}",
                                name=f"tmp{half}")
                nc.vector.tensor_add(
                    tmp[:], p_sb[:, half * 1024:half * 1024 + 512],
                    p_sb[:, half * 1024 + 512:(half + 1) * 1024])
                accs = acc01 if half == 0 else acc23
                if g == 0:
                    acc = accp.tile([128, 512], F32, tag=f"acc{half}",
                                    name=f"acc{half}")
                    nc.gpsimd.tensor_copy(acc[:], tmp[:])
                    accs[t] = acc
                else:
                    nc.gpsimd.tensor_add(accs[t][:], accs[t][:], tmp[:])

            def pv_pair(t, g, pair):
                """DoubleRow PV matmuls for j-blocks (4g+2*pair, 4g+2*pair+1)."""
                p_sb = p_tiles[(t, g)]
                pv = pvls[t]
                jb = 4 * g + 2 * pair
                p3d = p_sb[:, pair * 1024:(pair + 1) * 1024].rearrange(
                    "p (two n) -> p two n", two=2)
                vt3d = vt_sb[:, jb * C:(jb + 2) * C].rearrange(
                    "p (two c) -> p two c", two=2)
                for cb in range(2):
                    nc.tensor.matmul(
                        pv[cb][:],
                        vt3d[:, :, cb * 128:(cb + 1) * 128],
                        p3d,
                        start=(g == 0 and pair == 0),
                        stop=(g == NG - 1 and pair == 1),
                        perf_mode=DR)

            def epilogue_a(t):
                """After the last PV of slice t: fold accs, free pv banks."""
                acc_r = work.tile([128, 512], F32R, tag="acc_r", name="acc_r")
                nc.vector.tensor_add(acc_r[:], acc01[t][:], acc23[t][:])
                rps = ps_vt.tile([1, 512], F32, tag="vt", name="rps")
                nc.tensor.matmul(rps[:], ones_r[:], acc_r[:],
                                 start=True, stop=True)
                rinv = work.tile([1, 512], F32, tag="rinv", name="rinv")
                nc.vector.reciprocal_approx_fast(rinv[:], rps[:])
                rinv_r = work.tile([1, 512], F32R, tag="rinv_r", name="rinv_r")
                nc.vector.tensor_copy(rinv_r[:], rinv[:])
                pvs = []
                for cb in range(2):
                    p_cp = work.tile([128, 512], F32, tag=f"pvs{cb}",
                                     name=f"pvs{cb}")
                    nc.vector.tensor_copy(p_cp[:], pvls[t][cb][:])
                    pvs.append(p_cp)
                epi[t] = (rinv_r, pvs)

            def epilogue_b(t):
                rinv_r, pvs = epi.pop(t)
                rbc = ps_small.tile([128, 512], F32, tag="small", name="rbc")
                nc.tensor.matmul(rbc[:], onesrow_r[:], rinv_r[:],
                                 start=True, stop=True)
                for cb in range(2):
                    o_tmp = work.tile([128, 512], F32, tag="o_tmp",
                                      name="o_tmp")
                    nc.vector.tensor_mul(o_tmp[:], pvs[cb][:], rbc[:])
                    o_out = work.tile([128, 512], F32, tag="o_out",
                                      name="o_out")
                    nc.vector.tensor_add(
                        o_out[:], o_tmp[:],
                        xres_sb[:, cb * NQ + t * 512: cb * NQ + (t + 1) * 512])
                    nc.sync.dma_start(
                        out_ext[cb * 128:(cb + 1) * 128,
                                t * 512:(t + 1) * 512],
                        o_out[:])

            for i in range(len(pairs) + 2):
                cur = pairs[i] if i < len(pairs) else None
                prev = pairs[i - 1] if 1 <= i <= len(pairs) else None

                if cur is not None:
                    t, g = cur
                    p_tiles[cur] = pbuf.tile([128, 2048], F8P, tag="p",
                                             name="p_sb")
                    if cur == (0, 1):
                        k_proj(1)
                    if t == 0 and g >= 2:
                        vt_proj_pair(4 * g)
                        vt_proj_pair(4 * g + 2, pool=ps_small, tag="small")
                    scores_half(t, g, 0)
                if prev is not None:
                    if prev[1] == 0:
                        pvls[prev[0]] = [
                            ps_pv.tile([128, 512], F32, tag=f"pv{cb}",
                                       name=f"pv{cb}")
                            for cb in range(2)]
                    pv_pair(*prev, 0)
                if cur is not None:
                    scores_half(t, g, 1)
                    denom_half(t, g, 0)
                    if g == 3 and t < 3:
                        q_proj(t + 1)
                if prev is not None:
                    pv_pair(*prev, 1)
                    denom_half(*prev, 1)
                    if prev[1] == NG - 1:
                        epilogue_a(prev[0])
                if 2 <= i <= len(pairs) + 1:
                    tq, gq = pairs[i - 2]
                    if gq == NG - 1:
                        epilogue_b(tq)
    nc.compile()
    return nc


def _get_nc():
    if "nc" not in _cache:
        _cache["nc"] = _build()
    return _cache["nc"]


def _in_maps(x, wq, bq, wk, bk, wv, bv):
    wqt = np.ascontiguousarray(wq.T).astype(ml_dtypes.bfloat16)
    wkt = np.ascontiguousarray(wk.T).astype(ml_dtypes.bfloat16)
    wvt = np.ascontiguousarray(wv.T).astype(ml_dtypes.bfloat16)
    bq4 = np.ascontiguousarray(
        np.tile(np.asarray(bq, np.float32).reshape(D, 1), (4, 1)))
    bk4 = np.ascontiguousarray(
        np.tile(np.asarray(bk, np.float32).reshape(D, 1), (4, 1)))
    maps = []
    for core in range(NCORE):
        b, h = core // 2, core % 2
        xb = np.asarray(x[b], dtype=np.float32).reshape(C, N)
        if h == 1:
            xc = np.concatenate([xb[:, NQ:], xb[:, :NQ]], axis=1)
        else:
            xc = xb
        maps.append({
            "xb": np.ascontiguousarray(xc).astype(ml_dtypes.bfloat16),
            "xres": np.ascontiguousarray(
                xc[:, :NQ] + np.asarray(bv, np.float32).reshape(C, 1)),
            "wqt": wqt, "wkt": wkt, "wvt": wvt,
            "bq4": bq4, "bk4": bk4,
        })
    return maps


def _get_runner():
    """Build the SPMD graph once and cache a reusable jitted executable
    (run_bass_kernel_spmd re-jits per call, paying a full XLA compile)."""
    if "runner" in _cache:
        return _cache["runner"]
    import jax
    from jax.sharding import Mesh, PartitionSpec
    from jax.experimental.shard_map import shard_map
    from concourse import bass2jax, mybir as mb

    nc = _get_nc()
    bass2jax.install_neuronx_cc_hook()
    partition_name = (nc.partition_id_tensor.name
                      if nc.partition_id_tensor else None)
    in_names, out_names, out_avals, zero_shapes = [], [], [], []
    for alloc in nc.m.functions[0].allocations:
        if not isinstance(alloc, mb.MemoryLocationSet):
            continue
        name = alloc.memorylocations[0].name
        if alloc.kind == "ExternalInput":
            if name != partition_name:
                in_names.append(name)
        elif alloc.kind == "ExternalOutput":
            out_names.append(name)
            shape = tuple(alloc.tensor_shape)
            dtype = mb.dt.np(alloc.dtype)
            out_avals.append(jax.core.ShapedArray(shape, dtype))
            zero_shapes.append((shape, dtype))
    n_params = len(in_names)
    full_in_names = list(in_names) + list(out_names)
    if partition_name is not None:
        full_in_names.append(partition_name)
    donate = tuple(range(n_params, n_params + len(out_names)))

    def _body(*args):
        operands = list(args)
        if partition_name is not None:
            operands.append(bass2jax.partition_id_tensor())
        outs = bass2jax._bass_exec_p.bind(
            *operands,
            out_avals=tuple(out_avals),
            in_names=tuple(full_in_names),
            out_names=tuple(out_names),
            lowering_input_output_aliases=(),
            sim_require_finite=True,
            sim_require_nnan=True,
            nc=nc,
        )
        return tuple(outs)

    devices = jax.devices()[:NCORE]
    mesh = Mesh(np.asarray(devices), ("core",))
    in_specs = (PartitionSpec("core"),) * (n_params + len(out_names))
    out_specs = (PartitionSpec("core"),) * len(out_names)
    sharded = jax.jit(
        shard_map(_body, mesh=mesh, in_specs=in_specs, out_specs=out_specs,
                  check_rep=False),
        donate_argnums=donate, keep_unused=True)
    runner = (sharded, in_names, out_names, out_avals, zero_shapes)
    _cache["runner"] = runner
    return runner


def _run_fast(maps):
    sharded, in_names, out_names, out_avals, zero_shapes = _get_runner()
    concat_in = [
        np.concatenate([np.asarray(maps[c][name]) for c in range(NCORE)], axis=0)
        for name in in_names
    ]
    concat_zeros = [
        np.zeros((NCORE * s[0], *s[1:]), dt) for s, dt in zero_shapes
    ]
    out_arrs = sharded(*concat_in, *concat_zeros)
    return [
        {name: np.asarray(out_arrs[i]).reshape(NCORE, *out_avals[i].shape)[c]
         for i, name in enumerate(out_names)}
        for c in range(NCORE)
    ]


def _assemble(results):
    out = np.empty((4, C, N), dtype=np.float32)
    for core in range(NCORE):
        b, h = core // 2, core % 2
        out[b][:, h * NQ:(h + 1) * NQ] = results[core]["out"]
    return out.reshape(4, C, 64, 64)


def _run(inputs, trace=False, tmpdir=None):
    maps = _in_maps(**inputs)
    if trace:
        nc = _get_nc()
        res = run_bass_kernel_spmd(nc, maps, core_ids=list(range(NCORE)),
                                   trace=trace, tmpdir=tmpdir)
        return _assemble(res.results), res
    return _assemble(_run_fast(maps)), None


def kernel(**inputs):
    out, _ = _run(inputs)
    return out


# revision 8
# speedup vs baseline: 1.3932x; 1.3932x over previous
"""AttentionLayer Trainium2 kernel: 8-way SPMD (batch x query-half data parallel).

Per core (b = core//2, h = core%2), with x rotated so the core's query half
occupies columns 0..2047:
  k  = wk @ x + bk            [32, 4096]
  q  = wq @ x[:, :2048] + bq  [32, 2048]
  vT = x^T @ wv^T             [4096, 256]   (v transposed, born in [j, c] layout)
  S^T[j, i] = k[:, j]^T q[:, i]   -> P = exp(S^T - 8)  (fixed shift keeps P in
                                     fp8e5 range; shift cancels in the ratio)
  out[c, i] = (sum_j vT[j, c] P[j, i]) / (sum_j P[j, i]) + x[c, i]

The PV contraction (the dominant matmul, K=4096) runs in fp8 DoubleRow mode:
ACT exp reads each 2-bank score half [j-strip-pair x 512 i] with a 3D AP and
writes P as fp8e5 with the pair's two j-strips byte-interleaved ([p, 2i+u]),
so each DoubleRow matmul streams 2 contraction rows per cycle.  vT is
evacuated from PSUM as fp8e4.  The softmax denominator is folded into the PE
as a third DoubleRow matmul per pair (all-ones [128,2,1] stationary)
accumulating sum_j P into a [1,512] PSUM bank per t -- no DVE/GpSimd work
(fp8 elementwise ops measured ~3x slower than f32 on those engines).

Scores PSUM is split into two [128, 1024] ping-pong halves (scA/scB) so the
next block's score matmuls overlap the current block's exp (the WAR stall on
a single 4-bank score tile was the baseline's critical path).
"""
import numpy as np
import ml_dtypes

import concourse.bacc as bacc
import concourse.tile as tile
from concourse import mybir
from concourse.bass_utils import run_bass_kernel_spmd

F32 = mybir.dt.float32
F32R = mybir.dt.float32r
BF16 = mybir.dt.bfloat16
F8P = mybir.dt.float8e5    # P = exp(scores - OFF): wide range, 2-bit mantissa
F8V = mybir.dt.float8e4    # vT: |v| <~ 4, fine mantissa
AF = mybir.ActivationFunctionType
ALU = mybir.AluOpType
DR = mybir.MatmulPerfMode.DoubleRow

C = 256          # channels
D = 32           # q/k dim (C // 8)
N = 4096         # h*w
NQ = 2048        # queries per core
NCORE = 8
NG = 8           # score groups per slice (4 j-blocks each)
OFF = 8.0        # exp shift: max score ~13.1 -> max P ~ e^5.1 ~ 158 (fp8e5 ok)

_cache = {}


def _build():
    nc = bacc.Bacc(None, target_bir_lowering=False)
    xb_ext = nc.declare_dram_parameter("xb", [C, N], BF16, isOutput=False)
    xres_ext = nc.declare_dram_parameter("xres", [C, NQ], F32, isOutput=False)
    wqt_ext = nc.declare_dram_parameter("wqt", [C, D], BF16, isOutput=False)
    wkt_ext = nc.declare_dram_parameter("wkt", [C, D], BF16, isOutput=False)
    wvt_ext = nc.declare_dram_parameter("wvt", [C, C], BF16, isOutput=False)
    bq4_ext = nc.declare_dram_parameter("bq4", [128, 1], F32, isOutput=False)
    bk4_ext = nc.declare_dram_parameter("bk4", [128, 1], F32, isOutput=False)
    out_ext = nc.declare_dram_parameter("out", [C, NQ], F32, isOutput=True)

    with tile.TileContext(nc) as tc:
        with (
            tc.tile_pool(name="const", bufs=1) as const,
            tc.tile_pool(name="big", bufs=1) as big,
            tc.tile_pool(name="pbuf", bufs=3) as pbuf,
            tc.tile_pool(name="work", bufs=3) as work,
            tc.tile_pool(name="ps_scA", bufs=1, space="PSUM") as ps_scA,
            tc.tile_pool(name="ps_scB", bufs=1, space="PSUM") as ps_scB,
            tc.tile_pool(name="ps_pv", bufs=1, space="PSUM") as ps_pv,
            tc.tile_pool(name="ps_sh", bufs=1, space="PSUM") as ps_sh,
            tc.tile_pool(name="ps_dps", bufs=1, space="PSUM") as ps_dps,
        ):
            wqt_sb = const.tile([128, 2 * D], BF16)
            wkt_sb = const.tile([128, 2 * D], BF16)
            wvt_sb = const.tile([128, 2 * C], BF16)
            bq4_sb = const.tile([128, 1], F32)
            bk4_sb = const.tile([128, 1], F32)
            onesrow_f = const.tile([1, 128], F32)
            onesrow_r = const.tile([1, 128], F32R)
            negoff = const.tile([128, 1], F32)
            ones8 = const.tile([128, 32], F8P)   # DR ones stationary (stride 16)

            x_sb = big.tile([128, 2 * N], BF16)       # ci blocks side by side
            xres_sb = big.tile([128, 2 * NQ], F32)
            # k4: strip r (partitions 32r..32r+31) holds j-blocks 4g+r at
            # free g*128..(g+1)*128
            k4_sb = big.tile([128, 1024], BF16)
            # q4: strip r holds a full copy of q (slices side by side)
            q4_sb = big.tile([128, NQ], BF16)
            vt_sb = big.tile([128, 32 * C], F8V)      # [j%128, jb*256 + c]

            # critical-path DMAs first: q/k weights + the x chunks the first
            # q/k projections need; wvt + biases + rest of x on sync queue
            for ci in range(2):
                nc.scalar.dma_start(wqt_sb[:, ci * D:(ci + 1) * D],
                                    wqt_ext[ci * 128:(ci + 1) * 128, :])
                nc.scalar.dma_start(wkt_sb[:, ci * D:(ci + 1) * D],
                                    wkt_ext[ci * 128:(ci + 1) * 128, :])
            for s in range(4):
                for ci in range(2):
                    nc.scalar.dma_start(
                        x_sb[:, ci * N + s * 512: ci * N + (s + 1) * 512],
                        xb_ext[ci * 128:(ci + 1) * 128, s * 512:(s + 1) * 512])
            nc.sync.dma_start(bq4_sb[:], bq4_ext[:])
            nc.sync.dma_start(bk4_sb[:], bk4_ext[:])
            for ci in range(2):
                nc.sync.dma_start(wvt_sb[:, ci * C:(ci + 1) * C],
                                  wvt_ext[ci * 128:(ci + 1) * 128, :])
            nc.sync.dma_start(x_sb[:, 2048:4096], xb_ext[0:128, 2048:4096])
            nc.sync.dma_start(
                x_sb[:, N + 2048:2 * N], xb_ext[128:256, 2048:4096])
            nc.vector.memset(onesrow_f[:], 1.0)
            nc.vector.tensor_copy(onesrow_r[:], onesrow_f[:])
            nc.vector.memset(negoff[:], -OFF)
            nc.vector.memset(ones8[:], 1.0)

            def k_proj(gh):
                """Fill k4_sb[:, gh*512:(gh+1)*512] (j-blocks 16gh..16gh+15).

                Column-tiled: strip r gets blocks 4g+r, g in 4gh..4gh+3."""
                ps = ps_sh.tile([128, 512], F32, tag="sh", name="k_ps")
                for r in range(4):
                    for ci in range(2):
                        # rhs: x columns of blocks {4g+r : g in 4gh..4gh+3}
                        # block b at free offset b*128 = (4g+r)*128
                        base = ci * N + (16 * gh + r) * 128
                        rhs = x_sb[:, base: base + 13 * 128]
                        rhs = rhs.rearrange("p (g f) -> p g f", f=128)[:, 0:13:4, :]
                        nc.tensor.matmul(
                            ps[32 * r:32 * (r + 1), :],
                            wkt_sb[:, ci * D:(ci + 1) * D],
                            rhs,
                            start=(ci == 0), stop=(ci == 1),
                            tile_position=(0, 32 * r))
                nc.vector.tensor_scalar_add(
                    k4_sb[:, gh * 512:(gh + 1) * 512], ps[:], bk4_sb[:])

            def q_proj(t):
                """Fill q4_sb[:, t*512:(t+1)*512]: q slice replicated in 4 strips."""
                ps = ps_sh.tile([128, 512], F32, tag="sh", name="q_ps")
                for r in range(4):
                    for ci in range(2):
                        nc.tensor.matmul(
                            ps[32 * r:32 * (r + 1), :],
                            wqt_sb[:, ci * D:(ci + 1) * D],
                            x_sb[:, ci * N + t * 512: ci * N + (t + 1) * 512],
                            start=(ci == 0), stop=(ci == 1),
                            tile_position=(0, 32 * r))
                nc.vector.tensor_scalar_add(
                    q4_sb[:, t * 512:(t + 1) * 512], ps[:], bq4_sb[:])

            def vt_proj_pair(jb, pool, tag):
                """vT for j-blocks jb and jb+1 in one PSUM bank / one
                accumulation group (disjoint halves), one evacuation."""
                vps = pool.tile([128, 2 * C], F32, tag=tag, name="vt_ps")
                for u in range(2):
                    for ci in range(2):
                        nc.tensor.matmul(
                            vps[:, u * C:(u + 1) * C],
                            x_sb[:, ci * N + (jb + u) * 128:
                                 ci * N + (jb + u + 1) * 128],
                            wvt_sb[:, ci * C:(ci + 1) * C],
                            start=(u == 0 and ci == 0),
                            stop=(u == 1 and ci == 1))
                nc.vector.tensor_copy(vt_sb[:, jb * C:(jb + 2) * C], vps[:])

            pairs = [(t, g) for t in range(4) for g in range(NG)]
            pvls = {}
            dps = {}
            p_tiles = {}
            epi = {}

            def scores_half(t, g, half):
                """Score matmuls for strips (2*half, 2*half+1) into a 2-bank
                PSUM tile, then exp into the fp8 P tile with the two strips
                byte-interleaved ([p, 2i+u]) for DoubleRow streaming."""
                pool = ps_scA if half == 0 else ps_scB
                sc = pool.tile([128, 1024], F32, tag=f"sc{half}",
                               name=f"sc{half}")
                for rr in range(2):
                    r = 2 * half + rr
                    nc.tensor.matmul(
                        sc[:, rr * 512:(rr + 1) * 512],
                        k4_sb[32 * r:32 * (r + 1), g * 128:(g + 1) * 128],
                        q4_sb[32 * r:32 * (r + 1), t * 512:(t + 1) * 512],
                        start=True, stop=True,
                        tile_position=(32 * r, 0))
                p_sb = p_tiles[(t, g)]
                nc.scalar.activation(
                    p_sb[:, half * 1024:(half + 1) * 1024].rearrange(
                        "p (i u) -> p i u", u=2),
                    sc[:].rearrange("p (u i) -> p i u", u=2),
                    AF.Exp, bias=negoff[:])

            def pv_pair(t, g, pair):
                """DoubleRow PV + denominator matmuls for j-blocks
                (4g+2*pair, 4g+2*pair+1)."""
                p_sb = p_tiles[(t, g)]
                jb = 4 * g + 2 * pair
                p3d = p_sb[:, pair * 1024:(pair + 1) * 1024].rearrange(
                    "p (n two) -> p two n", two=2)
                vt3d = vt_sb[:, jb * C:(jb + 2) * C].rearrange(
                    "p (two c) -> p two c", two=2)
                first = (g == 0 and pair == 0)
                last = (g == NG - 1 and pair == 1)
                for cb in range(2):
                    nc.tensor.matmul(
                        pvls[t][cb][:],
                        vt3d[:, :, cb * 128:(cb + 1) * 128],
                        p3d,
                        start=first, stop=last,
                        perf_mode=DR)
                nc.tensor.matmul(
                    dps[t][:],
                    ones8[:].rearrange("p (two c) -> p two c", two=2)[:, :, 0:1],
                    p3d,
                    start=first, stop=last,
                    perf_mode=DR)

            def epilogue_a(t):
                """After the last PV of slice t: 1/denominator, free pv banks."""
                rinv = work.tile([1, 512], F32, tag="rinv", name="rinv")
                nc.vector.reciprocal_approx_fast(rinv[:], dps[t][:])
                rinv_r = work.tile([1, 512], F32R, tag="rinv_r", name="rinv_r")
                nc.vector.tensor_copy(rinv_r[:], rinv[:])
                pvs = []
                for cb in range(2):
                    p_cp = work.tile([128, 512], F32, tag=f"pvs{cb}",
                                     name=f"pvs{cb}")
                    nc.vector.tensor_copy(p_cp[:], pvls[t][cb][:])
                    pvs.append(p_cp)
                epi[t] = (rinv_r, pvs)

            def epilogue_b(t):
                rinv_r, pvs = epi.pop(t)
                rbc = ps_sh.tile([128, 512], F32, tag="sh", name="rbc")
                nc.tensor.matmul(rbc[:], onesrow_r[:], rinv_r[:],
                                 start=True, stop=True)
                for cb in range(2):
                    o_tmp = work.tile([128, 512], F32, tag="o_tmp",
                                      name="o_tmp")
                    nc.vector.tensor_mul(o_tmp[:], pvs[cb][:], rbc[:])
                    o_out = work.tile([128, 512], F32, tag="o_out",
                                      name="o_out")
                    nc.vector.tensor_add(
                        o_out[:], o_tmp[:],
                        xres_sb[:, cb * NQ + t * 512: cb * NQ + (t + 1) * 512])
                    nc.sync.dma_start(
                        out_ext[cb * 128:(cb + 1) * 128,
                                t * 512:(t + 1) * 512],
                        o_out[:])

            def stage1(t, g):
                p_tiles[(t, g)] = pbuf.tile([128, 2048], F8P, tag="p",
                                            name="p_sb")
                scores_half(t, g, 0)
                scores_half(t, g, 1)

            # ---- prologue: projections + first two blocks' scores, then all
            # vT pairs back-to-back rotating over 3 banks (sh, pv0, pv1)
            q_proj(0)
            k_proj(0)
            stage1(0, 0)
            vt_pools = [(ps_sh, "sh"), (ps_pv, "pv0"), (ps_pv, "pv1")]
            for idx, jb in enumerate(range(0, 8, 2)):
                pool, tag = vt_pools[idx % 3]
                vt_proj_pair(jb, pool, tag)
            stage1(0, 1)
            k_proj(1)
            for idx, jb in enumerate(range(8, 32, 2)):
                pool, tag = vt_pools[idx % 3]
                vt_proj_pair(jb, pool, tag)
            # xres is only needed at slice epilogues: emit late so these
            # transfers don't delay the critical-path x chunks
            for t in range(4):
                for ci in range(2):
                    nc.sync.dma_start(
                        xres_sb[:, ci * NQ + t * 512: ci * NQ + (t + 1) * 512],
                        xres_ext[ci * 128:(ci + 1) * 128, t * 512:(t + 1) * 512])

            # ---- main loop: stage1 (scores+exp) runs 2 blocks ahead of PV.
            # PE queue order per iteration matters: both PV pairs of the
            # previous block are ready at iteration start (their exps ran
            # an iteration ago), while scores of block cur=i+1 WAR-wait on
            # the exps of block i.  Interleave so the PE never idles behind
            # a not-yet-ready score matmul.
            for i in range(1, len(pairs) + 2):
                cur = pairs[i + 1] if i + 1 < len(pairs) else None
                prev = pairs[i - 1] if i <= len(pairs) else None

                if prev is not None and prev[1] == 0:
                    pvls[prev[0]] = [
                        ps_pv.tile([128, 512], F32, tag=f"pv{cb}",
                                   name=f"pv{cb}")
                        for cb in range(2)]
                    dps[prev[0]] = ps_dps.tile([1, 512], F32, tag="dps",
                                               name="dps")
                if prev is not None:
                    pv_pair(*prev, 0)
                if cur is not None:
                    p_tiles[cur] = pbuf.tile([128, 2048], F8P, tag="p",
                                             name="p_sb")
                    scores_half(*cur, 0)
                if prev is not None:
                    pv_pair(*prev, 1)
                if cur is not None:
                    scores_half(*cur, 1)
                    if cur[1] == 3 and cur[0] < 3:
                        q_proj(cur[0] + 1)
                if prev is not None and prev[1] == NG - 1:
                    epilogue_a(prev[0])
                if i >= 2 and i - 2 < len(pairs):
                    tq, gq = pairs[i - 2]
                    if gq == NG - 1:
                        epilogue_b(tq)
    nc.compile()
    return nc


def _get_nc():
    if "nc" not in _cache:
        _cache["nc"] = _build()
    return _cache["nc"]


def _in_maps(x, wq, bq, wk, bk, wv, bv):
    wqt = np.ascontiguousarray(wq.T).astype(ml_dtypes.bfloat16)
    wkt = np.ascontiguousarray(wk.T).astype(ml_dtypes.bfloat16)
    wvt = np.ascontiguousarray(wv.T).astype(ml_dtypes.bfloat16)
    bq4 = np.ascontiguousarray(
        np.tile(np.asarray(bq, np.float32).reshape(D, 1), (4, 1)))
    bk4 = np.ascontiguousarray(
        np.tile(np.asarray(bk, np.float32).reshape(D, 1), (4, 1)))
    maps = []
    for core in range(NCORE):
        b, h = core // 2, core % 2
        xb = np.asarray(x[b], dtype=np.float32).reshape(C, N)
        if h == 1:
            xc = np.concatenate([xb[:, NQ:], xb[:, :NQ]], axis=1)
        else:
            xc = xb
        maps.append({
            "xb": np.ascontiguousarray(xc).astype(ml_dtypes.bfloat16),
            "xres": np.ascontiguousarray(
                xc[:, :NQ] + np.asarray(bv, np.float32).reshape(C, 1)),
            "wqt": wqt, "wkt": wkt, "wvt": wvt,
            "bq4": bq4, "bk4": bk4,
        })
    return maps


def _get_runner():
    """Build the SPMD graph once and cache a reusable jitted executable
    (run_bass_kernel_spmd re-jits per call, paying a full XLA compile)."""
    if "runner" in _cache:
        return _cache["runner"]
    import jax
    from jax.sharding import Mesh, PartitionSpec
    from jax.experimental.shard_map import shard_map
    from concourse import bass2jax, mybir as mb

    nc = _get_nc()
    bass2jax.install_neuronx_cc_hook()
    partition_name = (nc.partition_id_tensor.name
                      if nc.partition_id_tensor else None)
    in_names, out_names, out_avals, zero_shapes = [], [], [], []
    for alloc in nc.m.functions[0].allocations:
        if not isinstance(alloc, mb.MemoryLocationSet):
            continue
        name = alloc.memorylocations[0].name
        if alloc.kind == "ExternalInput":
            if name != partition_name:
                in_names.append(name)
        elif alloc.kind == "ExternalOutput":
            out_names.append(name)
            shape = tuple(alloc.tensor_shape)
            dtype = mb.dt.np(alloc.dtype)
            out_avals.append(jax.core.ShapedArray(shape, dtype))
            zero_shapes.append((shape, dtype))
    n_params = len(in_names)
    full_in_names = list(in_names) + list(out_names)
    if partition_name is not None:
        full_in_names.append(partition_name)
    donate = tuple(range(n_params, n_params + len(out_names)))

    def _body(*args):
        operands = list(args)
        if partition_name is not None:
            operands.append(bass2jax.partition_id_tensor())
        outs = bass2jax._bass_exec_p.bind(
            *operands,
            out_avals=tuple(out_avals),
            in_names=tuple(full_in_names),
            out_names=tuple(out_names),
            lowering_input_output_aliases=(),
            sim_require_finite=True,
            sim_require_nnan=True,
            nc=nc,
        )
        return tuple(outs)

    devices = jax.devices()[:NCORE]
    mesh = Mesh(np.asarray(devices), ("core",))
    in_specs = (PartitionSpec("core"),) * (n_params + len(out_names))
    out_specs = (PartitionSpec("core"),) * len(out_names)
    sharded = jax.jit(
        shard_map(_body, mesh=mesh, in_specs=in_specs, out_specs=out_specs,
                  check_rep=False),
        donate_argnums=donate, keep_unused=True)
    runner = (sharded, in_names, out_names, out_avals, zero_shapes)
    _cache["runner"] = runner
    return runner


def _run_fast(maps):
    sharded, in_names, out_names, out_avals, zero_shapes = _get_runner()
    concat_in = [
        np.concatenate([np.asarray(maps[c][name]) for c in range(NCORE)], axis=0)
        for name in in_names
    ]
    concat_zeros = [
        np.zeros((NCORE * s[0], *s[1:]), dt) for s, dt in zero_shapes
    ]
    out_arrs = sharded(*concat_in, *concat_zeros)
    return [
        {name: np.asarray(out_arrs[i]).reshape(NCORE, *out_avals[i].shape)[c]
         for i, name in enumerate(out_names)}
        for c in range(NCORE)
    ]


def _assemble(results):
    out = np.empty((4, C, N), dtype=np.float32)
    for core in range(NCORE):
        b, h = core // 2, core % 2
        out[b][:, h * NQ:(h + 1) * NQ] = results[core]["out"]
    return out.reshape(4, C, 64, 64)


def _run(inputs, trace=False, tmpdir=None):
    maps = _in_maps(**inputs)
    if trace:
        nc = _get_nc()
        res = run_bass_kernel_spmd(nc, maps, core_ids=list(range(NCORE)),
                                   trace=trace, tmpdir=tmpdir)
        return _assemble(res.results), res
    return _assemble(_run_fast(maps)), None


def kernel(**inputs):
    out, _ = _run(inputs)
    return out
